# revision 7
# baseline (speedup 1.0000x reference)
"""Trainium2 Bass kernel for 16-head MHA (d_model=1024, B=2, S=2048, causal).

Sharding: tensor-parallel over heads — each of the 8 NeuronCores owns 2 heads
(column-split wq/wk/wv, row-split wo).  The partial output projections are
summed on the host (the canonical TP unshard), and the per-core attention
probabilities are written transposed ([kv, q]) and swapped back on the host.

Device dataflow (per core, all f32):
  qhT/khT [128, B*S] = wq_colsᵀ @ qᵀ           (host supplies qᵀ/kᵀ/vᵀ [D, B*S])
  vh      [s, 128]   = v @ wv_cols (+ bias via rank-1 ones matmul)
  scoresᵀ [kv, q]    = khTᵀ·qhT per head, causal blocks skipped,
                       diagonal 128x128 sub-blocks masked with -1e9 tile
  expᵀ               = exp(0.125·scoresᵀ)      (ScalarE, PSUM→SBUF)
  ctxuᵀ  [65, q]     = [vh | 1]ᵀ @ expᵀ        (ones column ⇒ softmax denom)
  recip → PE rank-1 broadcast [128, q] → normalize attenᵀ and ctxᵀ
  out_partial [B*S, 1024] = ctxᵀᵀ @ wo_rows
"""

import numpy as np

B, S, D_MODEL = 2, 2048, 1024
NUM_HEADS = 16
DEPTH = D_MODEL // NUM_HEADS  # 64
N_CORES = 8
HEADS_PER_CORE = NUM_HEADS // N_CORES  # 2
COLS = HEADS_PER_CORE * DEPTH  # 128 columns of wq/wk/wv per core

QC = 512   # q chunk (free dim of scores matmuls)
KB = 128   # kv block (partition dim of scoresT)

_PROGRAM_CACHE = {}

# debug: subset of {"attn", "ctx", "outproj"} — sections to emit (dev only)
_PHASES = {"attn", "ctx", "outproj"}


def _build_program(seq_len):
    """Build the per-core Bacc program. seq_len must be a multiple of 512."""
    import concourse.tile as tile
    from concourse import bacc, mybir

    f32 = mybir.dt.float32
    s_len = seq_len
    bs = B * s_len
    n_qc = s_len // QC          # q chunks per (b, h)
    n_kb = s_len // KB          # kv blocks per (b, h)
    kb_per_qc = QC // KB        # 4
    n_kchunk = D_MODEL // 128   # 8 contraction chunks for projections
    n_quarter = bs // 1024      # qhT/khT processed in [128, 1024] quarters
    n_sgroup = bs // 512        # vh processed in 512-row groups
    n_sblock = bs // 128

    nc = bacc.Bacc()

    qT_d = nc.declare_dram_parameter("qT", [D_MODEL, bs], f32, isOutput=False)
    kT_d = nc.declare_dram_parameter("kT", [D_MODEL, bs], f32, isOutput=False)
    vT_d = nc.declare_dram_parameter("vT", [D_MODEL, bs], f32, isOutput=False)
    wq_d = nc.declare_dram_parameter("wq", [D_MODEL, COLS], f32, isOutput=False)
    wk_d = nc.declare_dram_parameter("wk", [D_MODEL, COLS], f32, isOutput=False)
    wv_d = nc.declare_dram_parameter("wv", [D_MODEL, COLS], f32, isOutput=False)
    wo_d = nc.declare_dram_parameter("wo", [COLS, D_MODEL], f32, isOutput=False)
    bq_d = nc.declare_dram_parameter("bq", [COLS, 1], f32, isOutput=False)
    bk_d = nc.declare_dram_parameter("bk", [COLS, 1], f32, isOutput=False)
    bv_d = nc.declare_dram_parameter("bv", [1, COLS], f32, isOutput=False)
    maskT_d = nc.declare_dram_parameter("maskT", [KB, KB], f32, isOutput=False)

    attnT_d = nc.declare_dram_parameter(
        "attnT", [B, HEADS_PER_CORE, s_len, s_len], f32, isOutput=True)
    outp_d = nc.declare_dram_parameter("outp", [bs, D_MODEL], f32, isOutput=True)

    Exp = mybir.ActivationFunctionType.Exp
    scale = 1.0 / float(np.sqrt(DEPTH))

    with tile.TileContext(nc) as tc:
        with (
            tc.tile_pool(name="const", bufs=1) as const,
            tc.tile_pool(name="resid", bufs=1) as resid,
            tc.tile_pool(name="dmain", bufs=4) as dmain,
            tc.tile_pool(name="work", bufs=3) as work,
            tc.tile_pool(name="expp", bufs=n_kb + 2) as expp,
            tc.tile_pool(name="ps512", bufs=3, space="PSUM") as ps512,
            tc.tile_pool(name="pproj", bufs=2, space="PSUM") as pproj,
            tc.tile_pool(name="pctx", bufs=2, space="PSUM") as pctx,
            tc.tile_pool(name="pbc", bufs=1, space="PSUM") as pbc,
        ):
            # ---- constants ----
            wq_sb = const.tile([128, n_kchunk, 128], f32)
            wk_sb = const.tile([128, n_kchunk, 128], f32)
            wv_sb = const.tile([128, n_kchunk, 128], f32)
            nc.sync.dma_start(wq_sb[:], wq_d[:].rearrange("(c p) m -> p c m", p=128))
            nc.sync.dma_start(wk_sb[:], wk_d[:].rearrange("(c p) m -> p c m", p=128))
            nc.sync.dma_start(wv_sb[:], wv_d[:].rearrange("(c p) m -> p c m", p=128))
            wo_sb = const.tile([128, D_MODEL], f32)
            nc.sync.dma_start(wo_sb[:], wo_d[:])
            bq_sb = const.tile([COLS, 1], f32)
            bk_sb = const.tile([COLS, 1], f32)
            bv_sb = const.tile([1, COLS], f32)
            nc.sync.dma_start(bq_sb[:], bq_d[:])
            nc.sync.dma_start(bk_sb[:], bk_d[:])
            nc.sync.dma_start(bv_sb[:], bv_d[:])
            maskT = const.tile([KB, KB], f32)
            nc.sync.dma_start(maskT[:], maskT_d[:])
            ones = const.tile([1, 128], f32)
            nc.vector.memset(ones[:], 1.0)

            # ---- residents ----
            qhT = resid.tile([128, bs], f32)    # [head-col, b*S + s]
            khT = resid.tile([128, bs], f32)
            # vh + ones column, per (b, h, kv-block): 65 columns each
            vha = resid.tile([128, B * HEADS_PER_CORE * n_kb * 65], f32)
            ctxT = resid.tile([128, bs], f32)   # normalized ctx, transposed

            def vha_blk(b, h, i):
                return ((b * HEADS_PER_CORE + h) * n_kb + i) * 65

            # ones columns of vha
            nc.vector.memset(
                vha[:].rearrange("p (n c) -> p n c", c=65)[:, :, 64:65], 1.0)

            # ---- q/k projections: xhT[col, s] = wx_colsᵀ @ xT ----
            for xT_dram, w_sb, b_sb, dest in (
                (qT_d, wq_sb, bq_sb, qhT),
                (kT_d, wk_sb, bk_sb, khT),
            ):
                for t in range(n_quarter):
                    pq0 = pproj.tile([128, 512], f32, tag="pproj")
                    pq1 = pproj.tile([128, 512], f32, tag="pproj")
                    pqs = (pq0, pq1)
                    for kc in range(n_kchunk):
                        xin = dmain.tile([128, 1024], f32, tag="xin")
                        nc.sync.dma_start(
                            xin[:], xT_dram[kc * 128:(kc + 1) * 128,
                                            t * 1024:(t + 1) * 1024])
                        for n2 in range(2):
                            nc.tensor.matmul(
                                pqs[n2][:],
                                w_sb[:, kc, :],
                                xin[:, n2 * 512:(n2 + 1) * 512],
                                start=(kc == 0), stop=(kc == n_kchunk - 1))
                    # copy out with bias (Identity activation, per-partition bias)
                    for n2 in range(2):
                        nc.scalar.add(
                            dest[:, t * 1024 + n2 * 512: t * 1024 + (n2 + 1) * 512],
                            pqs[n2][:], b_sb[:])

            # ---- v projection: vh[s, col] = v @ wv_cols + bv ----
            for g in range(n_sgroup):
                pv = ps512.tile([128, 512], f32, tag="ps512")
                vins = []
                for kc in range(n_kchunk):
                    vin = dmain.tile([128, 512], f32, tag="vin", bufs=n_kchunk + 2)
                    nc.sync.dma_start(
                        vin[:], vT_d[kc * 128:(kc + 1) * 128,
                                     g * 512:(g + 1) * 512])
                    vins.append(vin)
                for j in range(4):
                    for kc in range(n_kchunk):
                        nc.tensor.matmul(
                            pv[:, j * 128:(j + 1) * 128],
                            vins[kc][:, j * 128:(j + 1) * 128],
                            wv_sb[:, kc, :],
                            start=(kc == 0), stop=False)
                    nc.tensor.matmul(
                        pv[:, j * 128:(j + 1) * 128],
                        ones[:, 0:128],
                        bv_sb[:],
                        start=False, stop=True)
                for j in range(4):
                    sb = g * 4 + j           # global s-block
                    b = sb // (s_len // 128)
                    i = sb % (s_len // 128)  # kv block within batch
                    for h in range(HEADS_PER_CORE):
                        nc.scalar.copy(
                            vha[:, vha_blk(b, h, i):vha_blk(b, h, i) + 64],
                            pv[:, j * 128 + h * 64:j * 128 + (h + 1) * 64])

            # ---- attention, fully transposed ----
            do_attn = "attn" in _PHASES
            do_ctx = "ctx" in _PHASES
            for b in range(B):
                for h in range(HEADS_PER_CORE):
                    if not do_attn:
                        break
                    hs = h * DEPTH  # row offset of this head in qhT/khT
                    for qc in range(n_qc):
                        nblk = kb_per_qc * (qc + 1)  # causal: kv blocks needed
                        if do_ctx:
                            ctxu = pctx.tile([65, 512], f32, tag="pctx")
                        etiles = []
                        for i in range(nblk):
                            jd = i - kb_per_qc * qc  # diag sub-block index
                            c0 = max(0, jd * 128)    # cols [0,c0) fully masked
                            ps = ps512.tile([128, 512], f32, tag="ps512")
                            nc.tensor.matmul(
                                ps[:, c0:512],
                                khT[hs:hs + DEPTH,
                                    b * s_len + i * 128: b * s_len + (i + 1) * 128],
                                qhT[hs:hs + DEPTH,
                                    b * s_len + qc * 512 + c0: b * s_len + (qc + 1) * 512],
                                start=True, stop=True)
                            if 0 <= jd < kb_per_qc:
                                nc.vector.tensor_add(
                                    ps[:, c0:c0 + 128], ps[:, c0:c0 + 128], maskT[:])
                            et = expp.tile([128, 512], f32, tag="et")
                            if c0 > 0:
                                nc.vector.memset(et[:, 0:c0], 0.0)
                            nc.scalar.activation(
                                et[:, c0:512], ps[:, c0:512], Exp, scale=scale)
                            if do_ctx:
                                nc.tensor.matmul(
                                    ctxu[:],
                                    vha[:, vha_blk(b, h, i):vha_blk(b, h, i) + 65],
                                    et[:],
                                    start=(i == 0), stop=(i == nblk - 1))
                            etiles.append((i, c0, et))
                        if not do_ctx:
                            for (i, c0, et) in etiles:
                                nc.sync.dma_start(
                                    attnT_d[b, h, i * 128:(i + 1) * 128,
                                            qc * 512:(qc + 1) * 512],
                                    et[:])
                            continue
                        recip = work.tile([1, 512], f32, tag="recip")
                        nc.vector.reciprocal(recip[:], ctxu[64:65, :])
                        bc = pbc.tile([128, 512], f32, tag="pbc")
                        nc.tensor.matmul(bc[:], ones[:, 0:128], recip[:],
                                         start=True, stop=True)
                        bcs = work.tile([128, 512], f32, tag="bcs")
                        nc.scalar.copy(bcs[:], bc[:])
                        nc.vector.tensor_mul(
                            ctxT[hs:hs + DEPTH,
                                 b * s_len + qc * 512: b * s_len + (qc + 1) * 512],
                            ctxu[0:64, :], bcs[0:64, :])
                        for (i, c0, et) in etiles:
                            nc.vector.tensor_mul(
                                et[:, c0:512], et[:, c0:512], bcs[:, c0:512])
                            nc.sync.dma_start(
                                attnT_d[b, h, i * 128:(i + 1) * 128,
                                        qc * 512:(qc + 1) * 512],
                                et[:])

            # ---- output projection: out_partial = ctx @ wo_rows ----
            if "outproj" not in _PHASES:
                n_sblock = 0
            for sb in range(n_sblock):
                for n2 in range(2):
                    po = ps512.tile([128, 512], f32, tag="ps512")
                    nc.tensor.matmul(
                        po[:], ctxT[:, sb * 128:(sb + 1) * 128],
                        wo_sb[:, n2 * 512:(n2 + 1) * 512],
                        start=True, stop=True)
                    ot = work.tile([128, 512], f32, tag="osb")
                    nc.scalar.copy(ot[:], po[:])
                    nc.sync.dma_start(
                        outp_d[sb * 128:(sb + 1) * 128,
                               n2 * 512:(n2 + 1) * 512], ot[:])

    nc.compile()
    return nc


def _get_program(seq_len):
    if seq_len not in _PROGRAM_CACHE:
        _PROGRAM_CACHE[seq_len] = _build_program(seq_len)
    return _PROGRAM_CACHE[seq_len]


def _make_maskT():
    """[kv, q] tile: -1e9 where kv > q (strictly below diagonal in T layout)."""
    m = np.zeros((KB, KB), np.float32)
    kv = np.arange(KB)[:, None]
    q = np.arange(KB)[None, :]
    m[kv > q] = -1e9
    return m


def _run_device(q, k, v, wq, bq, wk, bk, wv, bv, wo, seq_len, trace=False):
    from concourse.bass_utils import run_bass_kernel_spmd

    nc = _get_program(seq_len)
    bs = B * seq_len
    qT = np.ascontiguousarray(q.reshape(bs, D_MODEL).T)
    kT = np.ascontiguousarray(k.reshape(bs, D_MODEL).T)
    vT = np.ascontiguousarray(v.reshape(bs, D_MODEL).T)
    maskT = _make_maskT()

    in_maps = []
    for c in range(N_CORES):
        cs = c * COLS
        in_maps.append({
            "qT": qT, "kT": kT, "vT": vT,
            "wq": np.ascontiguousarray(wq[:, cs:cs + COLS]),
            "wk": np.ascontiguousarray(wk[:, cs:cs + COLS]),
            "wv": np.ascontiguousarray(wv[:, cs:cs + COLS]),
            "wo": np.ascontiguousarray(wo[cs:cs + COLS, :]),
            "bq": np.ascontiguousarray(bq[cs:cs + COLS]).reshape(COLS, 1),
            "bk": np.ascontiguousarray(bk[cs:cs + COLS]).reshape(COLS, 1),
            "bv": np.ascontiguousarray(bv[cs:cs + COLS]).reshape(1, COLS),
            "maskT": maskT,
        })
    res = run_bass_kernel_spmd(nc, in_maps, list(range(N_CORES)), trace=trace)
    return res


def _assemble(results, bo, seq_len):
    bs = B * seq_len
    atten = np.empty((B, NUM_HEADS, seq_len, seq_len), np.float32)
    out = np.zeros((bs, D_MODEL), np.float32)
    for c in range(N_CORES):
        r = results[c]
        atten[:, c * HEADS_PER_CORE:(c + 1) * HEADS_PER_CORE] = \
            r["attnT"].swapaxes(2, 3)
        out += r["outp"]
    out = out + bo[None, :].astype(np.float32)
    return out.reshape(B, seq_len, D_MODEL), atten


def _is_causal_mask(mask):
    m = np.asarray(mask)
    if m.shape != (1, 1, S, S):
        return False
    expect = 1.0 - np.tril(np.ones((S, S), np.float32))
    return np.array_equal(m.reshape(S, S).astype(np.float32), expect)


def _reference_fallback(q, k, v, mask, wq, bq, wk, bk, wv, bv, wo, bo):
    """Generic-mask numpy fallback (only used if the mask is not causal)."""
    def split_heads(x):
        b, s, _ = x.shape
        return x.reshape(b, s, NUM_HEADS, DEPTH).transpose(0, 2, 1, 3)

    qh = split_heads(q @ wq + bq)
    kh = split_heads(k @ wk + bk)
    vh = split_heads(v @ wv + bv)
    scale = 1.0 / np.sqrt(np.float32(DEPTH))
    logits = np.einsum("bhqd,bhkd->bhqk", qh, kh) * scale
    logits = logits + np.asarray(mask, np.float32) * np.float32(-1e9)
    logits -= logits.max(axis=-1, keepdims=True)
    e = np.exp(logits)
    atten = e / e.sum(axis=-1, keepdims=True)
    ctx = np.einsum("bhqk,bhkd->bhqd", atten, vh)
    ctx = ctx.transpose(0, 2, 1, 3).reshape(q.shape[0], -1, D_MODEL)
    out = ctx @ wo + bo
    return out.astype(np.float32), atten.astype(np.float32)


def kernel(q, k, v, mask, wq, bq, wk, bk, wv, bv, wo, bo):
    q = np.asarray(q, np.float32)
    k = np.asarray(k, np.float32)
    v = np.asarray(v, np.float32)
    wq = np.asarray(wq, np.float32)
    wk = np.asarray(wk, np.float32)
    wv = np.asarray(wv, np.float32)
    wo = np.asarray(wo, np.float32)
    bq = np.asarray(bq, np.float32)
    bk = np.asarray(bk, np.float32)
    bv = np.asarray(bv, np.float32)
    bo = np.asarray(bo, np.float32)

    if not _is_causal_mask(mask):
        return _reference_fallback(q, k, v, mask, wq, bq, wk, bk, wv, bv, wo, bo)

    res = _run_device(q, k, v, wq, bq, wk, bk, wv, bv, wo, S)
    return _assemble(res.results, bo, S)


# revision 10
# speedup vs baseline: 1.3046x; 1.3046x over previous
"""Trainium2 Bass kernel for 16-head MHA (d_model=1024, B=2, S=2048, causal).

Sharding: tensor-parallel over heads — each of the 8 NeuronCores owns 2 heads
(column-split wq/wk/wv, row-split wo).  The partial output projections are
summed on the host (the canonical TP unshard), and the per-core attention
probabilities are written transposed ([kv, q]) and swapped back on the host.

Device dataflow (per core, all f32):
  qhT/khT [128, B*S] = wq_colsᵀ @ qᵀ           (host supplies qᵀ/kᵀ/vᵀ [D, B*S])
  vh      [s, 128]   = v @ wv_cols (+ bias via rank-1 ones matmul)
  scoresᵀ [kv, q]    = khTᵀ·qhT per head, causal blocks skipped,
                       diagonal 128x128 sub-blocks masked with -1e9 tile
  expᵀ               = exp(0.125·scoresᵀ)      (ScalarE, PSUM→SBUF)
  ctxuᵀ  [65, q]     = [vh | 1]ᵀ @ expᵀ        (ones column ⇒ softmax denom)
  recip → PE rank-1 broadcast [128, q] → normalize attenᵀ and ctxᵀ
  out_partial [B*S, 1024] = ctxᵀᵀ @ wo_rows
"""

import numpy as np

B, S, D_MODEL = 2, 2048, 1024
NUM_HEADS = 16
DEPTH = D_MODEL // NUM_HEADS  # 64
N_CORES = 8
HEADS_PER_CORE = NUM_HEADS // N_CORES  # 2
COLS = HEADS_PER_CORE * DEPTH  # 128 columns of wq/wk/wv per core

QC = 512   # q chunk (free dim of scores matmuls)
KB = 128   # kv block (partition dim of scoresT)

_PROGRAM_CACHE = {}

# debug: subset of {"attn", "ctx", "outproj"} — sections to emit (dev only)
_PHASES = {"attn", "ctx", "outproj"}


def _build_program(seq_len):
    """Build the per-core Bacc program. seq_len must be a multiple of 512."""
    import concourse.tile as tile
    from concourse import bacc, mybir

    f32 = mybir.dt.float32
    f32r = mybir.dt.float32r
    s_len = seq_len
    bs = B * s_len
    n_qc = s_len // QC          # q chunks per (b, h)
    n_kb = s_len // KB          # kv blocks per (b, h)
    kb_per_qc = QC // KB        # 4
    n_kchunk = D_MODEL // 128   # 8 contraction chunks for projections
    n_quarter = bs // 1024      # qhT/khT processed in [128, 1024] quarters
    n_sgroup = bs // 512        # vh processed in 512-row groups
    n_sblock = bs // 128

    nc = bacc.Bacc()

    qT_d = nc.declare_dram_parameter("qT", [D_MODEL, bs], f32r, isOutput=False)
    kT_d = nc.declare_dram_parameter("kT", [D_MODEL, bs], f32r, isOutput=False)
    vT_d = nc.declare_dram_parameter("vT", [D_MODEL, bs], f32r, isOutput=False)
    wq_d = nc.declare_dram_parameter("wq", [D_MODEL, COLS], f32r, isOutput=False)
    wk_d = nc.declare_dram_parameter("wk", [D_MODEL, COLS], f32r, isOutput=False)
    wv_d = nc.declare_dram_parameter("wv", [D_MODEL, COLS], f32r, isOutput=False)
    wo_d = nc.declare_dram_parameter("wo", [COLS, D_MODEL], f32r, isOutput=False)
    bq_d = nc.declare_dram_parameter("bq", [COLS, 1], f32, isOutput=False)
    bk_d = nc.declare_dram_parameter("bk", [COLS, 1], f32, isOutput=False)
    bv_d = nc.declare_dram_parameter("bv", [1, COLS], f32r, isOutput=False)
    maskT_d = nc.declare_dram_parameter("maskT", [KB, KB], f32, isOutput=False)

    attnT_d = nc.declare_dram_parameter(
        "attnT", [B, HEADS_PER_CORE, s_len, s_len], f32r, isOutput=True)
    outp_d = nc.declare_dram_parameter("outp", [bs, D_MODEL], f32, isOutput=True)

    Exp = mybir.ActivationFunctionType.Exp
    scale = 1.0 / float(np.sqrt(DEPTH))

    with tile.TileContext(nc) as tc:
        with (
            tc.tile_pool(name="const", bufs=1) as const,
            tc.tile_pool(name="resid", bufs=1) as resid,
            tc.tile_pool(name="dmain", bufs=4) as dmain,
            tc.tile_pool(name="work", bufs=3) as work,
            tc.tile_pool(name="expp", bufs=n_kb + 2) as expp,
            tc.tile_pool(name="ps512", bufs=3, space="PSUM") as ps512,
            tc.tile_pool(name="pproj", bufs=2, space="PSUM") as pproj,
            tc.tile_pool(name="pctx", bufs=2, space="PSUM") as pctx,
            tc.tile_pool(name="pbc", bufs=1, space="PSUM") as pbc,
        ):
            # ---- constants ----
            wq_sb = const.tile([128, n_kchunk, 128], f32r)
            wk_sb = const.tile([128, n_kchunk, 128], f32r)
            wv_sb = const.tile([128, n_kchunk, 128], f32r)
            nc.sync.dma_start(wq_sb[:], wq_d[:].rearrange("(c p) m -> p c m", p=128))
            nc.sync.dma_start(wk_sb[:], wk_d[:].rearrange("(c p) m -> p c m", p=128))
            nc.sync.dma_start(wv_sb[:], wv_d[:].rearrange("(c p) m -> p c m", p=128))
            wo_sb = const.tile([128, D_MODEL], f32r)
            nc.sync.dma_start(wo_sb[:], wo_d[:])
            bq_sb = const.tile([COLS, 1], f32)
            bk_sb = const.tile([COLS, 1], f32)
            bv_sb = const.tile([1, COLS], f32r)
            nc.sync.dma_start(bq_sb[:], bq_d[:])
            nc.sync.dma_start(bk_sb[:], bk_d[:])
            nc.sync.dma_start(bv_sb[:], bv_d[:])
            maskT = const.tile([KB, KB], f32)
            nc.sync.dma_start(maskT[:], maskT_d[:])
            ones = const.tile([1, 128], f32r)
            ones_f = const.tile([1, 128], f32)
            nc.vector.memset(ones[:].bitcast(f32), 1.0)
            nc.vector.memset(ones_f[:], 1.0)

            # ---- residents ----
            qhT = resid.tile([128, bs], f32r)    # [head-col, b*S + s]
            khT = resid.tile([128, bs], f32r)
            # vh + ones column, per (b, h, kv-block): 65 columns each
            vha = resid.tile([128, B * HEADS_PER_CORE * n_kb * 65], f32r)
            ctxT = resid.tile([128, bs], f32r)  # normalized ctx, transposed

            def vha_blk(b, h, i):
                return ((b * HEADS_PER_CORE + h) * n_kb + i) * 65

            # ones columns of vha
            nc.vector.memset(
                vha[:].bitcast(f32).rearrange("p (n c) -> p n c", c=65)[:, :, 64:65],
                1.0)

            # ---- q/k projections: xhT[col, s] = wx_colsᵀ @ xT ----
            for xT_dram, w_sb, b_sb, dest in (
                (qT_d, wq_sb, bq_sb, qhT),
                (kT_d, wk_sb, bk_sb, khT),
            ):
                for t in range(n_quarter):
                    pq0 = pproj.tile([128, 512], f32, tag="pproj")
                    pq1 = pproj.tile([128, 512], f32, tag="pproj")
                    pqs = (pq0, pq1)
                    for kc in range(n_kchunk):
                        xin = dmain.tile([128, 1024], f32r, tag="xin")
                        nc.sync.dma_start(
                            xin[:], xT_dram[kc * 128:(kc + 1) * 128,
                                            t * 1024:(t + 1) * 1024])
                        for n2 in range(2):
                            nc.tensor.matmul(
                                pqs[n2][:],
                                w_sb[:, kc, :],
                                xin[:, n2 * 512:(n2 + 1) * 512],
                                start=(kc == 0), stop=(kc == n_kchunk - 1))
                    # copy out with bias (Identity activation, per-partition bias)
                    for n2 in range(2):
                        nc.scalar.add(
                            dest[:, t * 1024 + n2 * 512: t * 1024 + (n2 + 1) * 512],
                            pqs[n2][:], b_sb[:])

            # ---- v projection: vh[s, col] = v @ wv_cols + bv ----
            for g in range(n_sgroup):
                pv = ps512.tile([128, 512], f32, tag="ps512")
                vins = []
                for kc in range(n_kchunk):
                    vin = dmain.tile([128, 512], f32r, tag="vin", bufs=n_kchunk + 2)
                    nc.sync.dma_start(
                        vin[:], vT_d[kc * 128:(kc + 1) * 128,
                                     g * 512:(g + 1) * 512])
                    vins.append(vin)
                for j in range(4):
                    for kc in range(n_kchunk):
                        nc.tensor.matmul(
                            pv[:, j * 128:(j + 1) * 128],
                            vins[kc][:, j * 128:(j + 1) * 128],
                            wv_sb[:, kc, :],
                            start=(kc == 0), stop=False)
                    nc.tensor.matmul(
                        pv[:, j * 128:(j + 1) * 128],
                        ones[:, 0:128],
                        bv_sb[:],
                        start=False, stop=True)
                for j in range(4):
                    sb = g * 4 + j           # global s-block
                    b = sb // (s_len // 128)
                    i = sb % (s_len // 128)  # kv block within batch
                    for h in range(HEADS_PER_CORE):
                        nc.scalar.copy(
                            vha[:, vha_blk(b, h, i):vha_blk(b, h, i) + 64],
                            pv[:, j * 128 + h * 64:j * 128 + (h + 1) * 64])

            # ---- attention, fully transposed ----
            do_attn = "attn" in _PHASES
            do_ctx = "ctx" in _PHASES
            for b in range(B):
                for h in range(HEADS_PER_CORE):
                    if not do_attn:
                        break
                    hs = h * DEPTH  # row offset of this head in qhT/khT
                    for qc in range(n_qc):
                        nblk = kb_per_qc * (qc + 1)  # causal: kv blocks needed
                        if do_ctx:
                            ctxu = pctx.tile([65, 512], f32, tag="pctx")
                        etiles = []
                        for i in range(nblk):
                            jd = i - kb_per_qc * qc  # diag sub-block index
                            c0 = max(0, jd * 128)    # cols [0,c0) fully masked
                            ps = ps512.tile([128, 512], f32, tag="ps512")
                            nc.tensor.matmul(
                                ps[:, c0:512],
                                khT[hs:hs + DEPTH,
                                    b * s_len + i * 128: b * s_len + (i + 1) * 128],
                                qhT[hs:hs + DEPTH,
                                    b * s_len + qc * 512 + c0: b * s_len + (qc + 1) * 512],
                                start=True, stop=True)
                            if 0 <= jd < kb_per_qc:
                                nc.vector.tensor_add(
                                    ps[:, c0:c0 + 128], ps[:, c0:c0 + 128], maskT[:])
                            et = expp.tile([128, 512], f32r, tag="et")
                            if c0 > 0:
                                nc.vector.memset(et[:, 0:c0].bitcast(f32), 0.0)
                            nc.scalar.activation(
                                et[:, c0:512], ps[:, c0:512], Exp, scale=scale)
                            if do_ctx:
                                nc.tensor.matmul(
                                    ctxu[:],
                                    vha[:, vha_blk(b, h, i):vha_blk(b, h, i) + 65],
                                    et[:],
                                    start=(i == 0), stop=(i == nblk - 1))
                            etiles.append((i, c0, et))
                        if not do_ctx:
                            for (i, c0, et) in etiles:
                                nc.sync.dma_start(
                                    attnT_d[b, h, i * 128:(i + 1) * 128,
                                            qc * 512:(qc + 1) * 512],
                                    et[:])
                            continue
                        den_sb = work.tile([1, 512], f32, tag="den")
                        nc.scalar.copy(den_sb[:], ctxu[64:65, :])
                        recip = work.tile([1, 512], f32, tag="recip")
                        nc.vector.reciprocal_approx_fast(recip[:], den_sb[:])
                        bc = pbc.tile([128, 512], f32, tag="pbc")
                        nc.tensor.matmul(bc[:], ones_f[:, 0:128], recip[:],
                                         start=True, stop=True)
                        bcs = work.tile([128, 512], f32r, tag="bcs")
                        nc.scalar.copy(bcs[:], bc[:])
                        nc.vector.tensor_mul(
                            ctxT[hs:hs + DEPTH,
                                 b * s_len + qc * 512: b * s_len + (qc + 1) * 512],
                            ctxu[0:64, :], bcs[0:64, :])
                        for (i, c0, et) in etiles:
                            nc.vector.tensor_mul(
                                et[:, c0:512], et[:, c0:512], bcs[:, c0:512])
                            nc.sync.dma_start(
                                attnT_d[b, h, i * 128:(i + 1) * 128,
                                        qc * 512:(qc + 1) * 512],
                                et[:])

            # ---- output projection: out_partial = ctx @ wo_rows ----
            if "outproj" not in _PHASES:
                n_sblock = 0
            for sb in range(n_sblock):
                for n2 in range(2):
                    po = ps512.tile([128, 512], f32, tag="ps512")
                    nc.tensor.matmul(
                        po[:], ctxT[:, sb * 128:(sb + 1) * 128],
                        wo_sb[:, n2 * 512:(n2 + 1) * 512],
                        start=True, stop=True)
                    ot = work.tile([128, 512], f32, tag="osb")
                    nc.scalar.copy(ot[:], po[:])
                    nc.sync.dma_start(
                        outp_d[sb * 128:(sb + 1) * 128,
                               n2 * 512:(n2 + 1) * 512], ot[:])

    nc.compile()
    return nc


def _get_program(seq_len):
    if seq_len not in _PROGRAM_CACHE:
        _PROGRAM_CACHE[seq_len] = _build_program(seq_len)
    return _PROGRAM_CACHE[seq_len]


def _make_maskT():
    """[kv, q] tile: -1e9 where kv > q (strictly below diagonal in T layout)."""
    m = np.zeros((KB, KB), np.float32)
    kv = np.arange(KB)[:, None]
    q = np.arange(KB)[None, :]
    m[kv > q] = -1e9
    return m


def _run_device(q, k, v, wq, bq, wk, bk, wv, bv, wo, seq_len, trace=False):
    from concourse.bass_utils import run_bass_kernel_spmd

    nc = _get_program(seq_len)
    bs = B * seq_len
    qT = np.ascontiguousarray(q.reshape(bs, D_MODEL).T)
    kT = np.ascontiguousarray(k.reshape(bs, D_MODEL).T)
    vT = np.ascontiguousarray(v.reshape(bs, D_MODEL).T)
    maskT = _make_maskT()

    in_maps = []
    for c in range(N_CORES):
        cs = c * COLS
        in_maps.append({
            "qT": qT, "kT": kT, "vT": vT,
            "wq": np.ascontiguousarray(wq[:, cs:cs + COLS]),
            "wk": np.ascontiguousarray(wk[:, cs:cs + COLS]),
            "wv": np.ascontiguousarray(wv[:, cs:cs + COLS]),
            "wo": np.ascontiguousarray(wo[cs:cs + COLS, :]),
            "bq": np.ascontiguousarray(bq[cs:cs + COLS]).reshape(COLS, 1),
            "bk": np.ascontiguousarray(bk[cs:cs + COLS]).reshape(COLS, 1),
            "bv": np.ascontiguousarray(bv[cs:cs + COLS]).reshape(1, COLS),
            "maskT": maskT,
        })
    res = run_bass_kernel_spmd(nc, in_maps, list(range(N_CORES)), trace=trace)
    return res


def _assemble(results, bo, seq_len):
    bs = B * seq_len
    atten = np.empty((B, NUM_HEADS, seq_len, seq_len), np.float32)
    out = np.zeros((bs, D_MODEL), np.float32)
    for c in range(N_CORES):
        r = results[c]
        atten[:, c * HEADS_PER_CORE:(c + 1) * HEADS_PER_CORE] = \
            r["attnT"].swapaxes(2, 3)
        out += r["outp"]
    out = out + bo[None, :].astype(np.float32)
    return out.reshape(B, seq_len, D_MODEL), atten


def _is_causal_mask(mask):
    m = np.asarray(mask)
    if m.shape != (1, 1, S, S):
        return False
    expect = 1.0 - np.tril(np.ones((S, S), np.float32))
    return np.array_equal(m.reshape(S, S).astype(np.float32), expect)


def _reference_fallback(q, k, v, mask, wq, bq, wk, bk, wv, bv, wo, bo):
    """Generic-mask numpy fallback (only used if the mask is not causal)."""
    def split_heads(x):
        b, s, _ = x.shape
        return x.reshape(b, s, NUM_HEADS, DEPTH).transpose(0, 2, 1, 3)

    qh = split_heads(q @ wq + bq)
    kh = split_heads(k @ wk + bk)
    vh = split_heads(v @ wv + bv)
    scale = 1.0 / np.sqrt(np.float32(DEPTH))
    logits = np.einsum("bhqd,bhkd->bhqk", qh, kh) * scale
    logits = logits + np.asarray(mask, np.float32) * np.float32(-1e9)
    logits -= logits.max(axis=-1, keepdims=True)
    e = np.exp(logits)
    atten = e / e.sum(axis=-1, keepdims=True)
    ctx = np.einsum("bhqk,bhkd->bhqd", atten, vh)
    ctx = ctx.transpose(0, 2, 1, 3).reshape(q.shape[0], -1, D_MODEL)
    out = ctx @ wo + bo
    return out.astype(np.float32), atten.astype(np.float32)


def kernel(q, k, v, mask, wq, bq, wk, bk, wv, bv, wo, bo):
    q = np.asarray(q, np.float32)
    k = np.asarray(k, np.float32)
    v = np.asarray(v, np.float32)
    wq = np.asarray(wq, np.float32)
    wk = np.asarray(wk, np.float32)
    wv = np.asarray(wv, np.float32)
    wo = np.asarray(wo, np.float32)
    bq = np.asarray(bq, np.float32)
    bk = np.asarray(bk, np.float32)
    bv = np.asarray(bv, np.float32)
    bo = np.asarray(bo, np.float32)

    if not _is_causal_mask(mask):
        return _reference_fallback(q, k, v, mask, wq, bq, wk, bk, wv, bv, wo, bo)

    res = _run_device(q, k, v, wq, bq, wk, bk, wv, bv, wo, S)
    return _assemble(res.results, bo, S)
